# revision 4
# baseline (speedup 1.0000x reference)
"""DrBCNet GNN kernel for 8 TRN2 NeuronCores — sparse gather + ReduceScatter.

Strategy (src-sharded graph parallel):
  - Nodes dealt round-robin by out-degree to 8 cores (3750 each). Each core
    keeps ONLY its own h rows (bf16, [3840,128]) in local HBM each layer —
    no all-gather of embeddings.
  - Aggregation is computed from the source side: each core dma_gather's its
    own h rows once per outgoing edge (edges sorted by destination), then
    segment-sums 128-edge blocks into PSUM dst banks via tiny one-hot fp8
    matmuls (S matrices, SBUF-resident, reused across layers). The edge
    blocks live on a GLOBAL column grid (max-over-cores <=128 edges per
    block, <=64 dst columns) so the instruction stream is SPMD-uniform and
    only S / gather-index contents differ per core.
  - One bf16 ReduceScatter (feat-major [8,128,3840] -> [128,3840]) delivers
    each core the full aggregates for its own nodes; GRU gates, l2norm,
    cross-layer max and the decoder are purely node-local.
"""

import functools
import os

import numpy as np

CORES = 8
H = 128
L = 5
BANK = 512  # fp32 PSUM bank width
WMAX = 64  # max dst-column window per edge block
EBLK = 128  # edges per block (PE contraction width)
GROUP_BANKS = 8  # banks per dma_gather call
NORM_EPS_SQ = 1e-24


# ---------------------------------------------------------------- host planning
def _plan(edge_src, edge_dst, n_nodes):
    npc = n_nodes // CORES
    npc_pad = ((npc + 127) // 128) * 128
    ntiles = npc_pad // 128

    deg = np.bincount(edge_src, minlength=n_nodes)
    order = np.argsort(-deg, kind="stable")  # rank -> node
    gpos = np.empty(n_nodes, np.int64)
    gpos[order] = np.arange(n_nodes)
    owner = gpos % CORES
    pos = gpos // CORES
    order_per_core = [order[r::CORES] for r in range(CORES)]  # pos -> node id

    so = owner[edge_src]
    gcol = owner[edge_dst] * npc_pad + pos[edge_dst]

    # per-(core, dst column) edge counts -> global block grid
    ncol = CORES * npc_pad
    cnt = np.zeros((CORES, ncol), np.int64)
    np.add.at(cnt, (so, gcol), 1)
    assert cnt.max() <= EBLK, "single dst column exceeds one block per core"

    banks = []  # (gc0, width)
    for o in range(CORES):
        for c0 in range(0, npc_pad, BANK):
            banks.append((o * npc_pad + c0, min(BANK, npc_pad - c0)))

    blocks = []  # (bank_idx, w0_rel, wlen)
    nblk_bank = []
    for b, (g0, bw) in enumerate(banks):
        cc = cnt[:, g0 : g0 + bw]
        colsum = cc.sum(axis=0)
        c = 0
        nb0 = len(blocks)
        while c < bw:
            if colsum[c] == 0:
                c += 1
                continue
            acc = np.zeros(CORES, np.int64)
            w = 0
            while c + w < bw and w < WMAX:
                nxt = cc[:, c + w]
                if w > 0 and (acc + nxt > EBLK).any():
                    break
                acc += nxt
                w += 1
            blocks.append((b, c, w))
            c += w
        nblk_bank.append(len(blocks) - nb0)

    nb_tot = len(blocks)
    npad_e = nb_tot * EBLK

    # gather groups of GROUP_BANKS banks
    groups = []  # (blk0, nblk)
    bank_blk0 = np.cumsum([0] + nblk_bank)
    for g0b in range(0, len(banks), GROUP_BANKS):
        b0 = int(bank_blk0[g0b])
        b1 = int(bank_blk0[min(g0b + GROUP_BANKS, len(banks))])
        groups.append((b0, b1 - b0))
    nbg_max = max(nb for _, nb in groups)

    # per-core payloads: gather idxs + S one-hots
    import ml_dtypes

    block_lo = np.array([banks[b][0] + c for b, c, _ in blocks], np.int64)
    block_hi = np.array([banks[b][0] + c + w for b, c, w in blocks], np.int64)

    idx_all = []
    S_all = []
    for r in range(CORES):
        m = np.nonzero(so == r)[0]
        eg = gcol[m]
        es = pos[edge_src[m]]
        o2 = np.argsort(eg, kind="stable")
        eg = eg[o2]
        es = es[o2]
        lo = np.searchsorted(eg, block_lo)
        hi = np.searchsorted(eg, block_hi)
        assert (hi - lo).max() <= EBLK and (hi - lo).sum() == len(eg)
        idx = np.full(npad_e, npc, np.int16)  # dummy -> zero row
        S = np.zeros((EBLK, nb_tot * WMAX), np.float32)
        for j in range(nb_tot):
            n = hi[j] - lo[j]
            if n == 0:
                continue
            idx[j * EBLK : j * EBLK + n] = es[lo[j] : hi[j]]
            S[np.arange(n), j * WMAX + (eg[lo[j] : hi[j]] - block_lo[j])] = 1.0
        # wrap idx into 16 partitions, replicate x8 -> [128, npad_e/16]
        idxw = np.tile(idx.reshape(-1, 16).T, (8, 1))
        idx_all.append(np.ascontiguousarray(idxw))
        S_all.append(S.astype(ml_dtypes.float8_e4m3fn))

    return dict(
        npc=npc,
        npc_pad=npc_pad,
        ntiles=ntiles,
        banks=banks,
        blocks=blocks,
        nblk_bank=nblk_bank,
        groups=groups,
        nbg_max=nbg_max,
        nb_tot=nb_tot,
        npad_e=npad_e,
        order_per_core=order_per_core,
        owner=owner,
        pos=pos,
        idx_all=idx_all,
        S_all=S_all,
    )


# ---------------------------------------------------------------- bass program
def _build(meta):
    import concourse.bacc as bacc
    import concourse.mybir as mybir
    import concourse.tile as tile
    from concourse.masks import make_identity

    npc = meta["npc"]
    npc_pad = meta["npc_pad"]
    ntiles = meta["ntiles"]
    banks = meta["banks"]
    blocks = meta["blocks"]
    nblk_bank = meta["nblk_bank"]
    groups = meta["groups"]
    nbg_max = meta["nbg_max"]
    nb_tot = meta["nb_tot"]
    npad_e = meta["npad_e"]

    nbanks = len(banks)
    bank_blk0 = np.cumsum([0] + nblk_bank)

    f32 = mybir.dt.float32
    bf16 = mybir.dt.bfloat16
    i16 = mybir.dt.int16
    fp8 = mybir.dt.float8e4
    AF = mybir.ActivationFunctionType
    OP = mybir.AluOpType

    nc = bacc.Bacc(
        "TRN2", target_bir_lowering=False, debug=False, num_devices=CORES
    )

    # I/O
    xT_d = nc.dram_tensor("xT", [3, npc], f32, kind="ExternalInput")
    S_d = nc.dram_tensor("S", [128, nb_tot * WMAX], fp8, kind="ExternalInput")
    idx_d = nc.dram_tensor("idx", [128, npad_e // 16], i16, kind="ExternalInput")
    w1T_d = nc.dram_tensor("w1T", [3, 128], f32, kind="ExternalInput")
    b1_d = nc.dram_tensor("b1", [128, 1], f32, kind="ExternalInput")
    wih_d = nc.dram_tensor("wih", [128, 3 * H], bf16, kind="ExternalInput")
    whh_d = nc.dram_tensor("whh", [128, 3 * H], bf16, kind="ExternalInput")
    brz_d = nc.dram_tensor("brz", [128, 2], f32, kind="ExternalInput")
    bin_d = nc.dram_tensor("bin", [1, 128], bf16, kind="ExternalInput")
    bhn_d = nc.dram_tensor("bhn", [1, 128], bf16, kind="ExternalInput")
    w2T_d = nc.dram_tensor("w2T", [128, 128], bf16, kind="ExternalInput")
    b2_d = nc.dram_tensor("b2", [1, 128], bf16, kind="ExternalInput")
    out_d = nc.dram_tensor("out", [npc_pad, 128], f32, kind="ExternalOutput")

    h_tab = [nc.dram_tensor(f"htab{l}", [npc_pad, 128], bf16) for l in range(L)]
    agg_in = [
        nc.dram_tensor(f"aggin{l}", [CORES * 128, npc_pad], bf16) for l in range(L)
    ]
    agg_out = [
        nc.dram_tensor(f"aggout{l}", [128, npc_pad], bf16) for l in range(L)
    ]
    rs_groups = [list(range(CORES))]

    my_banks = [
        (i * BANK, min(BANK, npc - i * BANK)) for i in range((npc + BANK - 1) // BANK)
    ]

    with tile.TileContext(nc) as tc:
        import contextlib

        stack = contextlib.ExitStack()
        per = stack.enter_context(tc.tile_pool(name="per", bufs=1))

        def _T(shape, dtype, name):
            return per.tile(shape, dtype, name=name, tag=name)

        S_sb = _T([128, nb_tot * WMAX], fp8, "S_sb")
        idx_sb = _T([128, npad_e // 16], i16, "idx_sb")
        xT_sb = _T([3, npc], f32, "xT_sb")
        hT = _T([128, npc], f32, "hT")
        hTb = _T([128, npc], bf16, "hTb")
        hmaxTb = _T([128, npc], bf16, "hmaxTb")
        aggT = _T([128, npc_pad], bf16, "aggT")
        rows = _T([128, ntiles, 128], bf16, "rows")
        w1T_sb = _T([3, 128], f32, "w1T_sb")
        b1_sb = _T([128, 1], f32, "b1_sb")
        wih_sb = _T([128, 3 * H], bf16, "wih_sb")
        whh_sb = _T([128, 3 * H], bf16, "whh_sb")
        brz_sb = _T([128, 2], f32, "brz_sb")
        bin_sb = _T([1, 128], bf16, "bin_sb")
        bhn_sb = _T([1, 128], bf16, "bhn_sb")
        w2T_sb = _T([128, 128], bf16, "w2T_sb")
        b2_sb = _T([1, 128], bf16, "b2_sb")
        ones_col = _T([128, 1], bf16, "ones_col")
        ones_row = _T([1, BANK], bf16, "ones_row")
        onesk1 = _T([1, 128], bf16, "onesk1")
        zrow = _T([1, 128], bf16, "zrow")
        ident = _T([128, 128], bf16, "ident")
        eps_sb = _T([1, 1], f32, "eps_sb")

        hpool = stack.enter_context(tc.tile_pool(name="hpool", bufs=2))
        spool = stack.enter_context(tc.tile_pool(name="spool", bufs=4))
        tpool = stack.enter_context(tc.tile_pool(name="tpool", bufs=1))
        ps = stack.enter_context(tc.tile_pool(name="ps", bufs=8, space="PSUM"))

        # input loads
        nc.sync.dma_start(out=S_sb[:], in_=S_d[:])
        # Pool-issued: the ext-ISA gather can only wait DMASW sems on HW
        nc.gpsimd.dma_start(out=idx_sb[:], in_=idx_d[:])
        nc.sync.dma_start(out=xT_sb[:], in_=xT_d[:])
        nc.sync.dma_start(out=w1T_sb[:], in_=w1T_d[:])
        nc.sync.dma_start(out=b1_sb[:], in_=b1_d[:])
        nc.sync.dma_start(out=wih_sb[:], in_=wih_d[:])
        nc.sync.dma_start(out=whh_sb[:], in_=whh_d[:])
        nc.sync.dma_start(out=brz_sb[:], in_=brz_d[:])
        nc.sync.dma_start(out=bin_sb[:], in_=bin_d[:])
        nc.sync.dma_start(out=bhn_sb[:], in_=bhn_d[:])
        nc.sync.dma_start(out=w2T_sb[:], in_=w2T_d[:])
        nc.sync.dma_start(out=b2_sb[:], in_=b2_d[:])
        nc.vector.memset(eps_sb[:], NORM_EPS_SQ)
        nc.vector.memset(ones_col[:], 1.0)
        nc.vector.memset(ones_row[:], 1.0)
        nc.vector.memset(onesk1[:], 1.0)
        nc.vector.memset(zrow[:], 0.0)
        nc.vector.memset(rows[:, ntiles - 1, :], 0.0)
        make_identity(nc, ident[:])

        def norm_strip(l, b, s0, w):
            """hT[:, s0:s0+w] /= (sqrt(sum_f hT^2) + eps), per node column."""
            sq = tpool.tile([128, BANK], bf16, tag="sq")
            nc.vector.tensor_tensor(
                out=sq[:, :w], in0=hT[:, s0 : s0 + w], in1=hT[:, s0 : s0 + w],
                op=OP.mult,
            )
            ns_ps = ps.tile([1, BANK], f32, tag="ps", name=f"ns{l}_{b}")
            nc.tensor.matmul(
                out=ns_ps[:1, :w], lhsT=ones_col[:], rhs=sq[:, :w],
                start=True, stop=True,
            )
            srt = tpool.tile([1, BANK], f32, tag="srt")
            nc.scalar.activation(
                out=srt[:1, :w], in_=ns_ps[:1, :w], func=AF.Sqrt, bias=eps_sb[:1, :1]
            )
            inv_t = tpool.tile([1, BANK], bf16, tag="inv_t")
            with nc.allow_low_precision(reason="1/norm broadcast via bf16 matmul"):
                nc.vector.reciprocal(out=inv_t[:1, :w], in_=srt[:1, :w])
            bc_ps = ps.tile([128, BANK], f32, tag="ps", name=f"bc{l}_{b}")
            nc.tensor.matmul(
                out=bc_ps[:, :w], lhsT=onesk1[:1, :], rhs=inv_t[:1, :w],
                start=True, stop=True,
            )
            nc.vector.tensor_tensor(
                out=hT[:, s0 : s0 + w], in0=hT[:, s0 : s0 + w], in1=bc_ps[:, :w],
                op=OP.mult,
            )

        def store_rows(l):
            """hTb -> row-major bf16 table h_tab[l]."""
            for t in range(ntiles):
                wt = min(128, npc - t * 128)
                tp_ps = ps.tile([128, 128], bf16, tag="ps", name=f"tp{l}_{t}")
                nc.tensor.transpose(
                    out=tp_ps[:wt, :], in_=hTb[:, t * 128 : t * 128 + wt],
                    identity=ident[:],
                )
                nc.scalar.activation(
                    out=rows[:wt, t, :], in_=tp_ps[:wt, :], func=AF.Copy
                )
            dst = h_tab[l].ap().rearrange("(t p) f -> p t f", p=128)
            nc.gpsimd.dma_start(out=dst, in_=rows[:])

        # ---------------- encoder: hT = l2norm(relu(W1 @ x + b1))
        for b, (s0, w) in enumerate(my_banks):
            h0_ps = ps.tile([128, BANK], f32, tag="ps", name=f"enc{b}")
            nc.tensor.matmul(
                out=h0_ps[:, :w], lhsT=w1T_sb[:], rhs=xT_sb[:, s0 : s0 + w],
                start=True, stop=True,
            )
            nc.scalar.activation(
                out=hT[:, s0 : s0 + w], in_=h0_ps[:, :w], func=AF.Relu,
                bias=b1_sb[:, :1],
            )
            norm_strip("e", b, s0, w)
            nc.scalar.activation(
                out=hTb[:, s0 : s0 + w], in_=hT[:, s0 : s0 + w], func=AF.Copy
            )
            nc.vector.tensor_copy(
                out=hmaxTb[:, s0 : s0 + w], in_=hTb[:, s0 : s0 + w]
            )
        store_rows(0)

        # ---------------- message-passing layers
        for l in range(L):
            # src-side partial aggregation over the global dst space
            for g, (blk0, nblk) in enumerate(groups):
                hsrc = hpool.tile(
                    [128, nbg_max, 128], bf16, tag="hsrc", name=f"hsrc{l}_{g}"
                )
                # HW limit: a single dma_gather call crashes above 1024 idxs
                for k in range(0, nblk, 8):
                    nb = min(8, nblk - k)
                    nc.gpsimd.dma_gather(
                        hsrc[:, k : k + nb, :],
                        h_tab[l][:],
                        idx_sb[:, (blk0 + k) * 8 : (blk0 + k + nb) * 8],
                        nb * EBLK,
                        nb * EBLK,
                        H,
                    )
                for b in range(
                    g * GROUP_BANKS, min((g + 1) * GROUP_BANKS, nbanks)
                ):
                    g0, bw = banks[b]
                    o = b // (nbanks // CORES)
                    c0 = g0 - o * npc_pad
                    agg_ps = ps.tile([128, BANK], f32, tag="ps", name=f"agg{l}_{b}")
                    nc.tensor.matmul(
                        out=agg_ps[:, :bw], lhsT=zrow[:1, :], rhs=ones_row[:1, :bw],
                        start=True, stop=False,
                    )
                    for j in range(int(bank_blk0[b]), int(bank_blk0[b + 1])):
                        _, c, w = blocks[j]
                        nc.tensor.matmul(
                            out=agg_ps[:, c : c + w],
                            lhsT=hsrc[:, j - blk0, :],
                            rhs=S_sb[:, j * WMAX : j * WMAX + w],
                            start=False,
                            stop=False,
                            skip_group_check=True,
                        )
                    nc.tensor.matmul(
                        out=agg_ps[:, :bw], lhsT=zrow[:1, :], rhs=ones_row[:1, :bw],
                        start=False, stop=True,
                    )
                    stage = spool.tile([128, BANK], bf16, tag="stage")
                    if b % 2 == 0:
                        nc.scalar.activation(
                            out=stage[:, :bw], in_=agg_ps[:, :bw], func=AF.Copy
                        )
                    else:
                        nc.vector.tensor_copy(out=stage[:, :bw], in_=agg_ps[:, :bw])
                    nc.sync.dma_start(
                        out=agg_in[l][o * 128 : (o + 1) * 128, c0 : c0 + bw],
                        in_=stage[:, :bw],
                    )
            # reduce-scatter partial aggregates -> my nodes' aggregates
            nc.gpsimd.collective_compute(
                "ReduceScatter",
                OP.add,
                replica_groups=rs_groups,
                ins=[agg_in[l][:]],
                outs=[agg_out[l][:]],
            )
            nc.sync.dma_start(out=aggT[:], in_=agg_out[l][:])

            # GRU + l2norm + cross-layer max, per node bank
            for b, (s0, w) in enumerate(my_banks):
                ga = aggT[:, s0 : s0 + w]
                gh = hTb[:, s0 : s0 + w]

                def gate_ps(gi, name):
                    p = ps.tile([128, BANK], f32, tag="ps", name=name)
                    nc.tensor.matmul(
                        out=p[:, :w], lhsT=wih_sb[:, gi * H : (gi + 1) * H],
                        rhs=ga, start=True, stop=False,
                    )
                    nc.tensor.matmul(
                        out=p[:, :w], lhsT=whh_sb[:, gi * H : (gi + 1) * H],
                        rhs=gh, start=False, stop=True,
                    )
                    return p

                p_r = gate_ps(0, f"pr{l}_{b}")
                r_t = tpool.tile([128, BANK], f32, tag="r_t")
                nc.scalar.activation(
                    out=r_t[:, :w], in_=p_r[:, :w], func=AF.Sigmoid,
                    bias=brz_sb[:, 0:1],
                )
                p_z = gate_ps(1, f"pz{l}_{b}")
                z_t = tpool.tile([128, BANK], f32, tag="z_t")
                nc.scalar.activation(
                    out=z_t[:, :w], in_=p_z[:, :w], func=AF.Sigmoid,
                    bias=brz_sb[:, 1:2],
                )
                p_in = ps.tile([128, BANK], f32, tag="ps", name=f"pi{l}_{b}")
                nc.tensor.matmul(
                    out=p_in[:, :w], lhsT=bin_sb[:1, :], rhs=ones_row[:1, :w],
                    start=True, stop=False,
                )
                nc.tensor.matmul(
                    out=p_in[:, :w], lhsT=wih_sb[:, 2 * H : 3 * H], rhs=ga,
                    start=False, stop=True,
                )
                s_t = tpool.tile([128, BANK], f32, tag="s_t")
                nc.scalar.activation(out=s_t[:, :w], in_=p_in[:, :w], func=AF.Copy)
                p_hn = ps.tile([128, BANK], f32, tag="ps", name=f"ph{l}_{b}")
                nc.tensor.matmul(
                    out=p_hn[:, :w], lhsT=bhn_sb[:1, :], rhs=ones_row[:1, :w],
                    start=True, stop=False,
                )
                nc.tensor.matmul(
                    out=p_hn[:, :w], lhsT=whh_sb[:, 2 * H : 3 * H], rhs=gh,
                    start=False, stop=True,
                )
                t_t = tpool.tile([128, BANK], f32, tag="t_t")
                nc.scalar.activation(out=t_t[:, :w], in_=p_hn[:, :w], func=AF.Copy)
                # n = tanh(i_n + r * h_n)
                n_t = tpool.tile([128, BANK], f32, tag="n_t")
                nc.vector.tensor_tensor(
                    out=n_t[:, :w], in0=r_t[:, :w], in1=t_t[:, :w], op=OP.mult
                )
                nc.vector.tensor_tensor(
                    out=n_t[:, :w], in0=n_t[:, :w], in1=s_t[:, :w], op=OP.add
                )
                nc.scalar.activation(out=n_t[:, :w], in_=n_t[:, :w], func=AF.Tanh)
                # h' = n + z * (h - n)
                d_t = tpool.tile([128, BANK], f32, tag="d_t")
                nc.vector.tensor_tensor(
                    out=d_t[:, :w], in0=hT[:, s0 : s0 + w], in1=n_t[:, :w],
                    op=OP.subtract,
                )
                nc.vector.tensor_tensor(
                    out=d_t[:, :w], in0=d_t[:, :w], in1=z_t[:, :w], op=OP.mult
                )
                nc.vector.tensor_tensor(
                    out=hT[:, s0 : s0 + w], in0=d_t[:, :w], in1=n_t[:, :w], op=OP.add
                )
                norm_strip(l, b, s0, w)
                nc.scalar.activation(
                    out=hTb[:, s0 : s0 + w], in_=hT[:, s0 : s0 + w], func=AF.Copy
                )
                nc.vector.tensor_tensor(
                    out=hmaxTb[:, s0 : s0 + w], in0=hmaxTb[:, s0 : s0 + w],
                    in1=hTb[:, s0 : s0 + w], op=OP.max,
                )
            if l < L - 1:
                store_rows(l + 1)

        # ---------------- decoder: out = hmax @ W2.T + b2 (row layout)
        for t in range(ntiles):
            wt = min(128, npc - t * 128)
            o_ps = ps.tile([128, 128], f32, tag="ps", name=f"dec{t}")
            nc.tensor.matmul(
                out=o_ps[:wt, :], lhsT=onesk1[:1, :wt], rhs=b2_sb[:1, :],
                start=True, stop=False,
            )
            nc.tensor.matmul(
                out=o_ps[:wt, :], lhsT=hmaxTb[:, t * 128 : t * 128 + wt],
                rhs=w2T_sb[:], start=False, stop=True,
            )
            orow = tpool.tile([128, 128], f32, tag="orow")
            nc.scalar.activation(out=orow[:wt, :], in_=o_ps[:wt, :], func=AF.Copy)
            nc.sync.dma_start(
                out=out_d[t * 128 : t * 128 + wt, :], in_=orow[:wt, :]
            )
        stack.close()

    nc.compile()
    return nc


# ---------------------------------------------------------------- entry points
def _prep(inputs):
    import ml_dtypes

    x = np.asarray(inputs["x"], np.float32)
    edge_src = np.asarray(inputs["edge_src"], np.int64)
    edge_dst = np.asarray(inputs["edge_dst"], np.int64)
    n_nodes = x.shape[0]
    meta = _plan(edge_src, edge_dst, n_nodes)

    W1 = np.asarray(inputs["W1"], np.float32)
    b1 = np.asarray(inputs["b1"], np.float32)
    W_ih = np.asarray(inputs["W_ih"], np.float32)
    b_ih = np.asarray(inputs["b_ih"], np.float32)
    W_hh = np.asarray(inputs["W_hh"], np.float32)
    b_hh = np.asarray(inputs["b_hh"], np.float32)
    W2 = np.asarray(inputs["W2"], np.float32)
    b2 = np.asarray(inputs["b2"], np.float32)

    bf = ml_dtypes.bfloat16
    shared = dict(
        w1T=np.ascontiguousarray(W1.T),
        b1=np.ascontiguousarray(b1[:, None]),
        wih=np.ascontiguousarray(W_ih.T.astype(bf)),
        whh=np.ascontiguousarray(W_hh.T.astype(bf)),
        brz=np.ascontiguousarray(
            np.stack([b_ih[:H] + b_hh[:H], b_ih[H : 2 * H] + b_hh[H : 2 * H]], axis=1)
        ),
        bin=np.ascontiguousarray(b_ih[None, 2 * H :].astype(bf)),
        bhn=np.ascontiguousarray(b_hh[None, 2 * H :].astype(bf)),
        w2T=np.ascontiguousarray(W2.T.astype(bf)),
        b2=np.ascontiguousarray(b2[None, :].astype(bf)),
    )
    in_maps = []
    for r in range(CORES):
        xr = x[meta["order_per_core"][r]]
        in_maps.append(
            dict(
                xT=np.ascontiguousarray(xr.T),
                S=meta["S_all"][r],
                idx=meta["idx_all"][r],
                **shared,
            )
        )
    return meta, in_maps


def _assemble(meta, results, n_nodes):
    npc = meta["npc"]
    out = np.empty((n_nodes, 128), np.float32)
    for r in range(CORES):
        out[meta["order_per_core"][r]] = results[r]["out"][:npc]
    return out


@functools.lru_cache(maxsize=1)
def _get_compiled(key):
    meta, in_maps = _PENDING[key]
    nc = _build(meta)
    return nc, meta, in_maps


_PENDING = {}


def kernel(**inputs):
    x = np.asarray(inputs["x"])
    n_nodes = x.shape[0]
    meta, in_maps = _prep(inputs)
    key = hash(
        (
            n_nodes,
            np.asarray(inputs["edge_src"]).tobytes(),
            np.asarray(inputs["edge_dst"]).tobytes(),
        )
    )
    _PENDING[key] = (meta, in_maps)
    nc, meta, _ = _get_compiled(key)

    from concourse.bass_utils import run_bass_kernel_spmd

    trace = bool(int(os.environ.get("KERNEL_TRACE", "0")))
    res = run_bass_kernel_spmd(
        nc, in_maps, core_ids=list(range(CORES)), trace=trace
    )
    kernel.last_results = res
    return _assemble(meta, res.results, n_nodes)


# ---------------------------------------------------------------- numpy emulation
def _emulate(meta, inputs):
    """Pure-numpy emulation of the kernel dataflow (fp32 math, bf16 h table)
    to validate planning before building the Bass program."""
    import ml_dtypes

    x = np.asarray(inputs["x"], np.float32)
    n = x.shape[0]
    npc, npc_pad = meta["npc"], meta["npc_pad"]
    W1 = np.asarray(inputs["W1"], np.float32)
    b1 = np.asarray(inputs["b1"], np.float32)
    W_ih = np.asarray(inputs["W_ih"], np.float32)
    b_ih = np.asarray(inputs["b_ih"], np.float32)
    W_hh = np.asarray(inputs["W_hh"], np.float32)
    b_hh = np.asarray(inputs["b_hh"], np.float32)
    W2 = np.asarray(inputs["W2"], np.float32)
    b2 = np.asarray(inputs["b2"], np.float32)

    def bf16(a):
        return a.astype(ml_dtypes.bfloat16).astype(np.float32)

    def norm(h):
        return h / (np.sqrt((h * h).sum(-1, keepdims=True) + NORM_EPS_SQ))

    hs = []
    hmaxs = []
    for r in range(CORES):
        xr = x[meta["order_per_core"][r]]
        h = np.maximum(xr @ bf16(W1).T + b1, 0.0)
        h = norm(h)
        hs.append(h)
        hmaxs.append(h.copy())

    nb_tot, npad_e = meta["nb_tot"], meta["npad_e"]
    for l in range(L):
        # h tables (bf16, padded with zero rows)
        tabs = [
            np.concatenate(
                [bf16(hs[r]), np.zeros((npc_pad - npc, H), np.float32)], axis=0
            )
            for r in range(CORES)
        ]
        # per-core partial aggregation via gather + S
        agg_full = np.zeros((CORES, H, CORES * npc_pad), np.float32)
        for r in range(CORES):
            idxw = meta["idx_all"][r]
            idx = idxw[:16].T.reshape(-1)  # unwrap
            hsrc = tabs[r][idx]  # [npad_e, H]
            S = meta["S_all"][r].astype(np.float32)  # [128, nb*WMAX]
            for j, (b, c, w) in enumerate(meta["blocks"]):
                g0 = meta["banks"][b][0]
                blkh = bf16(hsrc[j * EBLK : (j + 1) * EBLK])  # [128, H]
                Sb = S[:, j * WMAX : j * WMAX + w]  # [128, w]
                agg_full[r, :, g0 + c : g0 + c + w] += blkh.T @ Sb
        # reduce-scatter (bf16 partials)
        agg_rs = bf16(agg_full).sum(axis=0)  # [H, CORES*npc_pad]
        for r in range(CORES):
            agg = bf16(agg_rs[:, r * npc_pad : r * npc_pad + npc].T)  # [npc, H]
            h = hs[r]
            gi = agg @ bf16(W_ih).T + b_ih
            gh = bf16(h) @ bf16(W_hh).T + b_hh
            i_r, i_z, i_n = np.split(gi, 3, -1)
            h_r, h_z, h_n = np.split(gh, 3, -1)
            rg = 1 / (1 + np.exp(-(i_r + h_r)))
            zg = 1 / (1 + np.exp(-(i_z + h_z)))
            ng = np.tanh(i_n + rg * h_n)
            h = (1 - zg) * ng + zg * h
            h = norm(h)
            hs[r] = h
            hmaxs[r] = np.maximum(hmaxs[r], bf16(h))

    out = np.empty((n, H), np.float32)
    for r in range(CORES):
        o = bf16(hmaxs[r]) @ bf16(W2).T + b2
        out[meta["order_per_core"][r]] = o[:npc]
    return out


if __name__ == "__main__":
    import time

    import jax

    import reference

    cpu = jax.devices("cpu")[0]
    with jax.default_device(cpu):
        inputs = {k: np.asarray(v) for k, v in reference.setup_inputs().items()}
        expected = np.asarray(reference.reference(**inputs))

    t0 = time.time()
    meta = _plan(
        np.asarray(inputs["edge_src"], np.int64),
        np.asarray(inputs["edge_dst"], np.int64),
        inputs["x"].shape[0],
    )
    print(f"plan in {time.time() - t0:.1f}s")
    print(
        f"nb_tot={meta['nb_tot']} npad_e={meta['npad_e']} "
        f"fill={600000 / CORES / meta['npad_e']:.3f} nbg_max={meta['nbg_max']} "
        f"S_bytes={meta['S_all'][0].nbytes} "
        f"gather_us_per_layer={meta['npad_e'] * 22.75 / 16 / 1000:.1f}"
    )
    t0 = time.time()
    out = _emulate(meta, inputs)
    print(f"emulate in {time.time() - t0:.1f}s")
    err = np.abs(out - expected)
    rel = err.max() / np.abs(expected).max()
    print(f"emulation rel err: {rel:.6e} (absmax={err.max():.3e})")
